# revision 1
# baseline (speedup 1.0000x reference)
"""Trainium2 Bass kernel for nn_Memory (scatter_memory): DNC-style memory module.

Computes, for N=1048576 memory slots, W=64, R=4 read heads:
  content_weighting = softmax(beta * cos_sim(memory, key))      (N,)
  retention         = prod_r (1 - read_weighting[:, r]*free_gate[r])
  usage             = (prev + write - prev*write) * retention
  allocation        = DNC allocation weighting (needs usage sorted ascending)
Returns np.stack([content, retention, usage, allocation]) -> (4, N) float32.

Strategy (8 NeuronCores, shard the N dimension):
  * Host shards rows N/8 per core and re-lays the memory matrix out as
    (W-packed, rows): partitions 0-63 = features of row-block A, 64-127 =
    features of row-block B.  fp32 values are split into fp16 hi+lo pairs
    (same total bytes as fp32) so the TensorEngine runs at full rate
    (fp32 matmul is 4x slower; fp16 streams 1 col/cycle).
  * Per core the TensorEngine computes row-dots against the pre-scaled key
    (key * beta / ||key||, fp16 hi/lo stationary columns) and row-sum-of-
    squares via a ones-matmul over DVE-squared fp16 tiles.  ScalarE derives
    rsqrt via Ln+Exp (one ACT table set) and the softmax numerators
    exp(beta*sim) with per-partition accumulated sums.  DVE does the
    retention/usage elementwise math.  Everything is DMA-bound.
  * Host glue: softmax normalization (sum of 256 partial sums), and the
    allocation weighting via a top-K trick: the ascending-sorted exclusive
    f32 cumprod of usage underflows to exact 0 within a few dozen terms, so
    only the K smallest usage slots can receive a nonzero allocation.  A
    full 1M global sort is unnecessary (with a full-argsort fallback if the
    cumprod somehow does not underflow).
"""

import os
import sys

import numpy as np

# concourse ships with the container (NIX_PYTHONPATH / sitecustomize); be
# defensive in case kernel.py is imported from a bare interpreter.
try:
    import concourse.bacc as bacc
except ImportError:  # pragma: no cover
    for _p in ("/opt/trn_rl_repo", "/root/.axon_site/_ro/trn_rl_repo"):
        if os.path.isdir(_p) and _p not in sys.path:
            sys.path.insert(0, _p)
    import concourse.bacc as bacc

import concourse.tile as tile
from concourse import mybir
from concourse.bass_utils import run_bass_kernel_spmd

F32 = mybir.dt.float32
F16 = mybir.dt.float16

N = 1048576
W = 64
R = 4
NCORES = 8
RPC = N // NCORES          # rows per core = 131072
HALF = RPC // 2            # rows per block = 65536
TILE_F = 4096              # rows per tile (per block)
NT = HALF // TILE_F        # 16 tiles
CHUNK = 512                # matmul moving free dim (one PSUM bank)
NCH = TILE_F // CHUNK      # 8 chunks per tile
EPS = 1e-8

# exported for test harness
LAST = {"exec_time_ns": None, "results": None}

_NC_CACHE = None


def _install_ntff_hook():
    """Register the axon NTFF profile hook if the image's antenv lacks it.

    Only needed when tracing (BASS_TRACE=1 / trace=True); harmless otherwise.
    """
    import types

    try:
        import antenv.axon_hooks  # noqa: F401

        return
    except ImportError:
        pass
    try:
        from trn_agent_boot.trn_boot import _ntff_profile_via_ctypes

        hook = _ntff_profile_via_ctypes("/opt/axon/libaxon_pjrt.so")
        mod = types.ModuleType("antenv.axon_hooks")
        mod.get_axon_ntff_profile_hook = lambda: hook
        mod.set_axon_ntff_profile_hook = lambda h: None
        sys.modules["antenv.axon_hooks"] = mod
        import antenv

        antenv.axon_hooks = mod
    except Exception:
        pass


def _build_nc():
    """Build the per-core Bass program (identical on all 8 cores)."""
    nc = bacc.Bacc(
        "TRN2",
        target_bir_lowering=False,
        debug=False,
        enable_asserts=False,
        num_devices=NCORES,
    )
    mt_ph = nc.dram_tensor("mt_ph", [128, HALF], F16, kind="ExternalInput").ap()
    mt_pl = nc.dram_tensor("mt_pl", [128, HALF], F16, kind="ExternalInput").ap()
    # 12 stationary variants (ti in 0..3 x pass in {ph,pl,sq}), each (128, 32)
    # with the key/ones columns shifted to offset 8*ti (zeros elsewhere) so a
    # 32-row matmul lands tile ti's rows at partition offset 8*ti inside a
    # 32-aligned PSUM region (PE col-group bases must be 0/32/64/96).
    skall = nc.dram_tensor("skall", [128, 12 * 32], F16, kind="ExternalInput").ap()
    negf = nc.dram_tensor("negf", [128, R], F32, kind="ExternalInput").ap()
    rwt = nc.dram_tensor("rwt", [128, R * 1024], F32, kind="ExternalInput").ap()
    prev = nc.dram_tensor("prev", [128, 1024], F32, kind="ExternalInput").ap()
    wr = nc.dram_tensor("wr", [128, 1024], F32, kind="ExternalInput").ap()

    p_out = nc.dram_tensor("p_out", [128, 1024], F32, kind="ExternalOutput").ap()
    ret_out = nc.dram_tensor("ret_out", [128, 1024], F32, kind="ExternalOutput").ap()
    use_out = nc.dram_tensor("use_out", [128, 1024], F32, kind="ExternalOutput").ap()
    esum_out = nc.dram_tensor("esum_out", [128, 1], F32, kind="ExternalOutput").ap()

    Ln = mybir.ActivationFunctionType.Ln
    Exp = mybir.ActivationFunctionType.Exp
    mult = mybir.AluOpType.mult
    add = mybir.AluOpType.add

    with tile.TileContext(nc) as tc:
        with (
            tc.tile_pool(name="const", bufs=1) as const,
            tc.tile_pool(name="mt", bufs=6) as mtp,
            tc.tile_pool(name="sq", bufs=3) as sqp,
            tc.tile_pool(name="work", bufs=1) as work,
            tc.tile_pool(name="ps", bufs=1, space="PSUM") as psp,
        ):
            sk_t = const.tile([128, 12 * 32], F16)
            nc.sync.dma_start(sk_t, skall)

            warm = const.tile([1, 1], F32)
            nc.vector.memset(warm, 1.0)

            # ---- heavy pass over the memory matrix ------------------------
            # PSUM layout: per tile t (g=t//4, ti=t%4), partitions
            # 32g+8ti .. 32g+8ti+8 hold
            #   [0:2] = ph@[khA,khB] + pl@[khA,khB]  (dot hi+lo, PE-summed)
            #   [2:4] = ph @ [klA,klB]               (dot key-lo part)
            #   [4:6] = sq @ [onesA,onesB]           (row sum of squares)
            #   [6:8] = unused (zero)
            # Each matmul is M=32 with shifted stationary columns; the four
            # tiles of a group accumulate into the same 32-row region
            # (start on ti==0/ph, stop on ti==3/sq).
            ps = psp.tile([128, TILE_F], F32)
            # res2 gathers results with natural row layout: partition p' =
            # global_row // 1024, free blocks [dots_hi | dots_klo | sumsq]
            # each 1024 wide -> finishing ops use all 128 DVE/ACT lanes and
            # p_out is a natural (128,1024) reshape.
            res2 = work.tile([128, 3 * 1024], F32)
            ret_col = None
            for t in range(NT):
                g, ti = divmod(t, 4)
                base = 32 * g
                ph_t = mtp.tile([128, TILE_F], F16, tag="ph")
                nc.sync.dma_start(ph_t, mt_ph[:, t * TILE_F : (t + 1) * TILE_F])
                pl_t = mtp.tile([128, TILE_F], F16, tag="pl")
                nc.sync.dma_start(pl_t, mt_pl[:, t * TILE_F : (t + 1) * TILE_F])
                sq_t = sqp.tile([128, TILE_F], F16, tag="sq")
                nc.vector.tensor_mul(sq_t, ph_t, ph_t)
                for p_i, mv in ((0, ph_t), (2, sq_t), (1, pl_t)):
                    v = 3 * ti + p_i
                    lhs = sk_t[:, v * 32 : (v + 1) * 32]
                    for c in range(NCH):
                        cs = slice(c * CHUNK, (c + 1) * CHUNK)
                        nc.tensor.matmul(
                            ps[base : base + 32, cs], lhs, mv[:, cs],
                            start=(ti == 0 and p_i == 0),
                            stop=(ti == 3 and p_i == 1),
                            tile_position=(0, base),
                        )
                if t == 2:
                    # Warm the ACT Ln/Exp spline tables (input chained to this
                    # tile's squares so the ~1.3us PSEUDO_LOAD_ACT_FUNC_SET
                    # TDRAM DMAs don't race the first big input loads, yet
                    # still overlap the main loop instead of the tail).
                    nc.scalar.activation(warm, sq_t[0:1, 0:1], Ln, bias=1.0)
                    nc.scalar.activation(warm, sq_t[0:1, 0:1], Exp, scale=-1.0)
                if t == 1:
                    # retention/usage: independent small work, emitted here so
                    # it overlaps the heavy loop instead of the tail
                    ret_col = _retention_usage(
                        nc, tc, const, work, negf, rwt, prev, wr, ret_out,
                        use_out, mult, add,
                    )
                if t == NT - 1:
                    # re-warm the Ln table set (Ln and Exp live in different
                    # sets; the Exp set from the warm-up evicted Ln's) so the
                    # tail's real Ln doesn't eat a ~1.3us table load
                    nc.scalar.activation(warm, sq_t[0:1, 0:1], Ln, bias=1.0)
            # PSUM -> SBUF once (a mid-loop drain would serialize against the
            # next group's matmuls via PSUM bank-conflict tracking), then six
            # SBUF->SBUF DMAs permute tile-major partitions into res2's
            # natural row layout.
            res = work.tile([128, TILE_F], F32)
            nc.scalar.copy(res[:, 0 : TILE_F // 2], ps[:, 0 : TILE_F // 2])
            nc.vector.tensor_copy(res[:, TILE_F // 2 :], ps[:, TILE_F // 2 :])
            resv = res.rearrange("(t r) (q j) -> t r q j", r=8, j=1024)
            for b in range(2):
                for blk in range(3):
                    dst = res2[64 * b : 64 * b + 64, blk * 1024 : (blk + 1) * 1024]
                    eng = nc.scalar if (2 * blk + b) % 2 else nc.sync
                    eng.dma_start(dst, resv[:, 2 * blk + b, :, :])

            kh = res2[:, 0:1024]
            kl = res2[:, 1024:2048]
            ssq = res2[:, 2048:3072]
            nc.vector.tensor_add(kh, kh, kl)       # dots = hi + key-lo
            # rsqrt(ssq) = exp(-0.5*ln(ssq)); ACT Rsqrt is banned (accuracy)
            nc.scalar.activation(ssq, ssq, Ln)
            nc.scalar.activation(ssq, ssq, Exp, scale=-0.5)
            nc.vector.tensor_mul(kh, kh, ssq)      # beta*sim (key pre-scaled)
            esum = work.tile([128, 1], F32)
            nc.scalar.activation(kh, kh, Exp, accum_out=esum)
            nc.scalar.dma_start(p_out, kh)
            nc.scalar.dma_start(esum_out, esum)

    nc.compile()
    return nc


def _retention_usage(nc, tc, const, work, negf, rwt, prev, wr, ret_out, use_out,
                     mult, add):
    """retention = prod_r (1 - w_r*f_r); usage = (p + w - p*w) * retention."""
    F32 = mybir.dt.float32
    nf_t = const.tile([128, R], F32)
    nc.scalar.dma_start(nf_t, negf)
    rw_t = work.tile([128, R * 1024], F32)
    nc.scalar.dma_start(rw_t, rwt)
    for h in range(R):
        hs = slice(h * 1024, (h + 1) * 1024)
        # in-place: a_h = (w_h * -f_h) + 1
        nc.vector.tensor_scalar(
            rw_t[:, hs], rw_t[:, hs], nf_t[:, h : h + 1], 1.0,
            op0=mult, op1=add,
        )
    h0, h1 = rw_t[:, 0:1024], rw_t[:, 1024:2048]
    h2, h3 = rw_t[:, 2048:3072], rw_t[:, 3072:4096]
    nc.vector.tensor_mul(h0, h0, h1)
    nc.vector.tensor_mul(h2, h2, h3)
    nc.vector.tensor_mul(h0, h0, h2)       # retention in rw_t[:, :1024]
    nc.scalar.dma_start(ret_out, h0)

    pv_t = work.tile([128, 1024], F32)
    nc.scalar.dma_start(pv_t, prev)
    wr_t = work.tile([128, 1024], F32)
    nc.scalar.dma_start(wr_t, wr)
    us_t = work.tile([128, 1024], F32)
    nc.vector.tensor_add(us_t, pv_t, wr_t)
    nc.vector.tensor_mul(pv_t, pv_t, wr_t)     # prev*wr in place
    nc.vector.tensor_sub(us_t, us_t, pv_t)
    nc.vector.tensor_mul(us_t, us_t, h0)
    nc.scalar.dma_start(use_out, us_t)
    return h0


def _get_nc():
    global _NC_CACHE
    if _NC_CACHE is None:
        _NC_CACHE = _build_nc()
    return _NC_CACHE


def kernel(
    desired_content,
    memory,
    key_strength,
    free_gate,
    read_weighting,
    previous_usage,
    write_weighting,
):
    desired_content = np.asarray(desired_content, np.float32)
    memory = np.asarray(memory, np.float32)
    key_strength = np.asarray(key_strength, np.float32)
    free_gate = np.asarray(free_gate, np.float32)
    read_weighting = np.asarray(read_weighting, np.float32)
    previous_usage = np.asarray(previous_usage, np.float32)
    write_weighting = np.asarray(write_weighting, np.float32)

    # ---- host prep: shared small tensors ---------------------------------
    kn = max(float(np.linalg.norm(desired_content)), EPS)
    scale = np.float32(float(key_strength[0]) / kn)
    skey = (desired_content * scale).astype(np.float32)
    khh = skey.astype(np.float16)
    kll = (skey - khh.astype(np.float32)).astype(np.float16)
    skall = np.zeros((128, 12, 32), np.float16)
    for ti in range(4):
        o = 8 * ti
        skall[0:64, 3 * ti + 0, o + 0] = khh
        skall[64:128, 3 * ti + 0, o + 1] = khh
        skall[0:64, 3 * ti + 0, o + 2] = kll
        skall[64:128, 3 * ti + 0, o + 3] = kll
        skall[0:64, 3 * ti + 1, o + 0] = khh
        skall[64:128, 3 * ti + 1, o + 1] = khh
        skall[0:64, 3 * ti + 2, o + 4] = 1.0
        skall[64:128, 3 * ti + 2, o + 5] = 1.0
    skall = np.ascontiguousarray(skall.reshape(128, 12 * 32))
    negf = np.tile(-free_gate.astype(np.float32), (128, 1))

    # ---- host prep: per-core shards --------------------------------------
    in_maps = []
    mt = np.empty((128, HALF), np.float32)
    for c in range(NCORES):
        sl = slice(c * RPC, (c + 1) * RPC)
        shard = memory[sl]
        mt[:64] = shard[:HALF].T
        mt[64:] = shard[HALF:].T
        ph = mt.astype(np.float16)
        pl = (mt - ph.astype(np.float32)).astype(np.float16)
        rw = read_weighting[sl]
        rwt = np.empty((128, R * 1024), np.float32)
        for h in range(R):
            rwt[:, h * 1024 : (h + 1) * 1024] = rw[:, h].reshape(128, 1024)
        in_maps.append(
            {
                "mt_ph": ph,
                "mt_pl": pl,
                "skall": skall,
                "negf": negf,
                "rwt": rwt,
                "prev": np.ascontiguousarray(previous_usage[sl]).reshape(128, 1024),
                "wr": np.ascontiguousarray(write_weighting[sl]).reshape(128, 1024),
            }
        )

    # ---- run on the 8 NeuronCores ----------------------------------------
    trace = os.environ.get("BASS_TRACE", "") not in ("", "0")
    if trace:
        _install_ntff_hook()
    nc = _get_nc()
    reps = int(os.environ.get("BASS_REPEAT", "1"))
    times = []
    for rep in range(reps):
        res = run_bass_kernel_spmd(
            nc,
            in_maps,
            core_ids=list(range(NCORES)),
            trace=trace,
            tmpdir=(os.environ.get("BASS_TRACE_DIR") or None) if reps == 1 else None,
        )
        if res.exec_time_ns is not None:
            times.append(res.exec_time_ns)
    LAST["exec_time_ns"] = min(times) if times else None
    LAST["exec_times"] = times
    LAST["results"] = res

    # ---- gather / unshard -------------------------------------------------
    pnum = np.concatenate([r["p_out"].reshape(-1) for r in res.results])
    retention = np.concatenate([r["ret_out"].reshape(-1) for r in res.results])
    usage = np.concatenate([r["use_out"].reshape(-1) for r in res.results])
    esum = np.concatenate([r["esum_out"].reshape(-1) for r in res.results])
    S = np.sum(esum, dtype=np.float32)
    content = (pnum / S).astype(np.float32)

    allocation = _allocation_weighting(usage)

    return np.stack([content, retention, usage, allocation]).astype(np.float32)


def _allocation_weighting(usage: np.ndarray) -> np.ndarray:
    """Faithful f32 replica of the reference allocation computation.

    ref:  idx = argsort(usage) (stable ascending); s = usage[idx]
          alloc_sorted = (1 - s[max(j-1,0)]) * prod_{i<j} s[i]
          allocation[idx] = alloc_sorted
    The exclusive cumprod of ascending f32 values in [0,1) underflows to
    exact 0 within a few dozen terms, so only the K smallest slots matter.
    """
    n = usage.shape[0]
    K = min(1024, n)
    cand = np.argpartition(usage, K - 1)[:K]
    order = np.lexsort((cand, usage[cand]))  # by value, ties by index (stable)
    sidx = cand[order]
    s = usage[sidx].astype(np.float32)
    excl = np.empty(K, np.float32)
    excl[0] = np.float32(1.0)
    np.cumprod(s[:-1], dtype=np.float32, out=excl[1:])
    if K < n and excl[-1] != 0.0:
        # cumprod did not underflow within K terms: fall back to full sort
        sidx = np.argsort(usage, kind="stable")
        s = usage[sidx].astype(np.float32)
        excl = np.concatenate(
            [[np.float32(1.0)], np.cumprod(s[:-1], dtype=np.float32)]
        ).astype(np.float32)
    shifted = np.concatenate([s[:1], s[:-1]])
    alloc_sorted = ((np.float32(1.0) - shifted) * excl).astype(np.float32)
    allocation = np.zeros(n, np.float32)
    allocation[sidx] = alloc_sorted
    return allocation



# revision 9
# speedup vs baseline: 1.4276x; 1.4276x over previous
"""Trainium2 Bass kernel for nn_Memory (scatter_memory): DNC-style memory module.

Computes, for N=1048576 memory slots, W=64, R=4 read heads:
  content_weighting = softmax(beta * cos_sim(memory, key))      (N,)
  retention         = prod_r (1 - read_weighting[:, r]*free_gate[r])
  usage             = (prev + write - prev*write) * retention
  allocation        = DNC allocation weighting (needs usage sorted ascending)
Returns np.stack([content, retention, usage, allocation]) -> (4, N) float32.

Strategy (8 NeuronCores, shard the N dimension; DMA-bandwidth bound):
  * The rel-err gate is 2e-2, so all large tensors move as fp16 (the
    memory matrix as a single fp16 copy, not the fp32-exact hi+lo pair):
    ~19.2 MB of HBM traffic per core vs 38.3 MB for the exact variant.
  * Memory is re-laid as (W-packed, rows): partitions 0-63 = features of
    row-block A, 64-127 = block B.  TensorE computes row-dots against the
    pre-scaled key (key * beta / ||key||, fp16) at 1 col/cycle.
  * Row sums-of-squares use an fp8(e4m3) DoubleRow matmul: squares are
    quantized to fp8 on ACT/DVE, and the (128, 2, n) k-tiled matmul
    contracts 256 values/col so the ssq pass runs at 2 rows per PE col
    (half the columns AND double rate) - TensorE stays under the DMA
    roofline.
  * ScalarE derives rsqrt via Ln+Exp and the softmax numerators
    exp(beta*sim) with per-partition accumulated sums.  DVE does the
    retention/usage elementwise math in fp16.
  * Host glue: softmax normalization (sum of partial sums), and the
    allocation weighting via a top-K trick (ascending-sorted exclusive
    f32 cumprod of usage underflows to exact 0 within a few dozen terms,
    so only the K smallest usage slots can receive nonzero allocation).
    The K candidate usages are recomputed exactly on host from the fp32
    inputs so the allocation ordering matches the reference bit-exactly.
"""

import os
import sys

import numpy as np

try:
    import concourse.bacc as bacc
except ImportError:  # pragma: no cover
    for _p in ("/opt/trn_rl_repo", "/root/.axon_site/_ro/trn_rl_repo"):
        if os.path.isdir(_p) and _p not in sys.path:
            sys.path.insert(0, _p)
    import concourse.bacc as bacc

import ml_dtypes
import concourse.tile as tile
from concourse import mybir
from concourse.bass_utils import run_bass_kernel_spmd

F32 = mybir.dt.float32
F16 = mybir.dt.float16
F8 = mybir.dt.float8e4

N = 1048576
W = 64
R = 4
NCORES = 8
RPC = N // NCORES          # rows per core = 131072
HALF = RPC // 2            # rows per block = 65536
TILE_F = 4096              # rows per tile (per block)
NT = HALF // TILE_F        # 16 tiles
CHUNK = 512                # matmul moving free dim (one PSUM bank)
NCH = TILE_F // CHUNK      # 8 chunks per tile
EPS = 1e-8

# exported for test harness
LAST = {"exec_time_ns": None, "results": None}

_NC_CACHE = None


def _install_ntff_hook():
    """Register the axon NTFF profile hook if the image's antenv lacks it."""
    import types

    try:
        import antenv.axon_hooks  # noqa: F401

        return
    except ImportError:
        pass
    try:
        from trn_agent_boot.trn_boot import _ntff_profile_via_ctypes

        hook = _ntff_profile_via_ctypes("/opt/axon/libaxon_pjrt.so")
        mod = types.ModuleType("antenv.axon_hooks")
        mod.get_axon_ntff_profile_hook = lambda: hook
        mod.set_axon_ntff_profile_hook = lambda h: None
        sys.modules["antenv.axon_hooks"] = mod
        import antenv

        antenv.axon_hooks = mod
    except Exception:
        pass


def _build_nc():
    """Build the per-core Bass program (identical on all 8 cores)."""
    nc = bacc.Bacc(
        "TRN2",
        target_bir_lowering=False,
        debug=False,
        enable_asserts=False,
        num_devices=NCORES,
    )
    mt = nc.dram_tensor("mt", [128, HALF], F16, kind="ExternalInput").ap()
    # dots stationary: 16 variants (tile t), each (128, 32) fp16 with the
    # scaled key at cols {2t: block A (partitions 0-63), 2t+1: block B}.
    skd = nc.dram_tensor("skd", [128, NT * 32], F16, kind="ExternalInput").ap()
    # ssq stationary: 8 variants (t%8), each (128, 2, 32) fp8; variant v has
    # indicator cols 4v+m with m = 2*(p//64) + j  (j = DoubleRow k-tile).
    sk8 = nc.dram_tensor("sk8", [128, 2, 8 * 32], F8, kind="ExternalInput").ap()
    negf = nc.dram_tensor("negf", [128, R], F32, kind="ExternalInput").ap()
    rwt = nc.dram_tensor("rwt", [128, R * 1024], F16, kind="ExternalInput").ap()
    prev = nc.dram_tensor("prev", [128, 1024], F16, kind="ExternalInput").ap()
    wr = nc.dram_tensor("wr", [128, 1024], F16, kind="ExternalInput").ap()

    p_out = nc.dram_tensor("p_out", [128, 1024], F16, kind="ExternalOutput").ap()
    ret_out = nc.dram_tensor("ret_out", [128, 1024], F16, kind="ExternalOutput").ap()
    use_out = nc.dram_tensor("use_out", [128, 1024], F16, kind="ExternalOutput").ap()
    esum_out = nc.dram_tensor("esum_out", [128, 1], F32, kind="ExternalOutput").ap()

    Ln = mybir.ActivationFunctionType.Ln
    Exp = mybir.ActivationFunctionType.Exp
    mult = mybir.AluOpType.mult
    add = mybir.AluOpType.add
    DR = mybir.MatmulPerfMode.DoubleRow

    with tile.TileContext(nc) as tc:
        with (
            tc.tile_pool(name="const", bufs=1) as const,
            tc.tile_pool(name="mt", bufs=4) as mtp,
            tc.tile_pool(name="sq", bufs=3) as sqp,
            tc.tile_pool(name="work", bufs=1) as work,
            tc.tile_pool(name="ps", bufs=1, space="PSUM") as psp,
        ):
            skd_t = const.tile([128, NT * 32], F16)
            nc.sync.dma_start(skd_t, skd)
            sk8_t = const.tile([128, 2, 8 * 32], F8)
            nc.sync.dma_start(sk8_t, sk8)

            # PSUM layout (128, 4096) f32.  DoubleRow matmul outputs are only
            # legal at PSUM partition base 0, so:
            #   rows  0-31, cols    0-2048 : ssq tiles 0-7; tile t -> rows
            #               4(t%8)+m, m in {A lo-half, A hi-half, B lo-half,
            #               B hi-half}, col = position within half-tile.
            #   rows  0-31, cols 2048-4096 : ssq tiles 8-15.
            #   rows 32-63, cols    0-4096 : dots; tile t -> rows
            #               {32+2t: blkA, 32+2t+1: blkB}, col = pos in tile.
            ps = psp.tile([128, TILE_F], F32)
            ret_h0 = None
            for t in range(NT):
                ph = mtp.tile([128, TILE_F], F16, tag="ph")
                nc.sync.dma_start(ph, mt[:, t * TILE_F : (t + 1) * TILE_F])
                # fp8 squares, viewed (128, 2, 2048): j selects the half-tile
                sq = sqp.tile([128, 2, TILE_F // 2], F8, tag="sq")
                nc.scalar.square(sq[:, 0, :], ph[:, 0 : TILE_F // 2])
                nc.vector.tensor_mul(
                    sq[:, 1, :], ph[:, TILE_F // 2 :], ph[:, TILE_F // 2 :]
                )
                # dots: 8 chunk-matmuls, 512 cols each
                lhs_d = skd_t[:, t * 32 : (t + 1) * 32]
                for c in range(NCH):
                    cs = slice(c * CHUNK, (c + 1) * CHUNK)
                    nc.tensor.matmul(
                        ps[32:64, cs], lhs_d, ph[:, cs],
                        start=(t == 0),
                        stop=(t == NT - 1),
                        tile_position=(0, 32),
                    )
                # ssq: 4 DoubleRow chunk-matmuls, rhs free (2, 512) -> 512 out
                g, v = divmod(t, 8)
                coff = 2048 * g
                lhs_q = sk8_t[:, :, v * 32 : (v + 1) * 32]
                for c in range(NCH // 2):
                    nc.tensor.matmul(
                        ps[0:32, coff + c * CHUNK : coff + (c + 1) * CHUNK],
                        lhs_q, sq[:, :, c * CHUNK : (c + 1) * CHUNK],
                        start=(v == 0),
                        stop=(v == 7),
                        perf_mode=DR,
                        tile_position=(0, 0),
                    )
                if t == 1:
                    # retention/usage: independent small work, overlapped
                    ret_h0 = _retention_usage(
                        nc, tc, const, work, negf, rwt, prev, wr, ret_out,
                        use_out, mult, add,
                    )

            # ---- tail: drain PSUM, permute to natural layout, finish -------
            res = work.tile([128, TILE_F], F32)
            # split drains ACT/DVE; PSUM reads may not cross 32-row groups
            nc.scalar.copy(res[32:64, 0:2048], ps[32:64, 0:2048])     # dots L
            nc.vector.tensor_copy(res[32:64, 2048:], ps[32:64, 2048:])  # dots R
            nc.scalar.copy(res[0:32, 0:2048], ps[0:32, 0:2048])       # ssq G0
            nc.vector.tensor_copy(res[0:32, 2048:], ps[0:32, 2048:])  # ssq G1

            # natural layout res2 (128, 2048): [0:1024] dots, [1024:2048] ssq
            # partition p' = (global row)//1024 (blk A: 0-63, blk B: 64-127)
            res2 = work.tile([128, 2048], F32)
            # dots: src partitions 32+2t+b, free (q=4, j=1024)
            resd = res.rearrange("(t r) (q j) -> t r q j", r=2, j=1024)
            nc.sync.dma_start(res2[0:64, 0:1024], resd[16:32, 0, :, :])
            nc.scalar.dma_start(res2[64:128, 0:1024], resd[16:32, 1, :, :])
            # ssq: src partition 4u+m (u = t%8), free (k j): k = 2g + n//1024
            # dst partition 32g+4u+2*(m%2)+(n//1024)  (+64 for B, m>=2)
            resq = res.rearrange("(u m) (k j) -> u m k j", m=4, j=1024)
            res2r = res2.rearrange("(u b) (c j) -> u b c j", b=4, j=1024)
            for g in range(2):
                for m in range(4):
                    for k in range(2):
                        u0 = 16 * (m // 2) + 8 * g
                        dst = res2r[u0 : u0 + 8, 2 * (m % 2) + k, 1, :]
                        src = resq[0:8, m, 2 * g + k, :]
                        eng = nc.sync if (m + k) % 2 else nc.scalar
                        eng.dma_start(dst, src)

            kh = res2[:, 0:1024]
            ssq = res2[:, 1024:2048]
            # rsqrt(ssq) = exp(-0.5*ln(ssq)); ACT Rsqrt is banned (accuracy)
            nc.scalar.activation(ssq, ssq, Ln)
            nc.scalar.activation(ssq, ssq, Exp, scale=-0.5)
            nc.vector.tensor_mul(kh, kh, ssq)      # beta*sim (key pre-scaled)
            p16 = work.tile([128, 1024], F16)
            esum = work.tile([128, 1], F32)
            nc.scalar.activation(p16, kh, Exp, accum_out=esum)
            nc.scalar.dma_start(p_out, p16)
            nc.scalar.dma_start(esum_out, esum)

    nc.compile()
    return nc


def _retention_usage(nc, tc, const, work, negf, rwt, prev, wr, ret_out, use_out,
                     mult, add):
    """retention = prod_r (1 - w_r*f_r); usage = (p + w - p*w) * retention."""
    nf_t = const.tile([128, R], F32)
    nc.scalar.dma_start(nf_t, negf)
    rw_t = work.tile([128, R * 1024], F16)
    nc.scalar.dma_start(rw_t, rwt)
    for h in range(R):
        hs = slice(h * 1024, (h + 1) * 1024)
        # in-place: a_h = (w_h * -f_h) + 1
        nc.vector.tensor_scalar(
            rw_t[:, hs], rw_t[:, hs], nf_t[:, h : h + 1], 1.0,
            op0=mult, op1=add,
        )
    h0, h1 = rw_t[:, 0:1024], rw_t[:, 1024:2048]
    h2, h3 = rw_t[:, 2048:3072], rw_t[:, 3072:4096]
    nc.vector.tensor_mul(h0, h0, h1)
    nc.vector.tensor_mul(h2, h2, h3)
    nc.vector.tensor_mul(h0, h0, h2)       # retention in rw_t[:, :1024]
    nc.scalar.dma_start(ret_out, h0)

    pv_t = work.tile([128, 1024], F16)
    nc.scalar.dma_start(pv_t, prev)
    wr_t = work.tile([128, 1024], F16)
    nc.scalar.dma_start(wr_t, wr)
    us_t = work.tile([128, 1024], F16)
    nc.vector.tensor_add(us_t, pv_t, wr_t)
    nc.vector.tensor_mul(pv_t, pv_t, wr_t)     # prev*wr in place
    nc.vector.tensor_sub(us_t, us_t, pv_t)
    nc.vector.tensor_mul(us_t, us_t, h0)
    nc.scalar.dma_start(use_out, us_t)
    return h0


def _get_nc():
    global _NC_CACHE
    if _NC_CACHE is None:
        _NC_CACHE = _build_nc()
    return _NC_CACHE


def kernel(
    desired_content,
    memory,
    key_strength,
    free_gate,
    read_weighting,
    previous_usage,
    write_weighting,
):
    desired_content = np.asarray(desired_content, np.float32)
    memory = np.asarray(memory, np.float32)
    key_strength = np.asarray(key_strength, np.float32)
    free_gate = np.asarray(free_gate, np.float32)
    read_weighting = np.asarray(read_weighting, np.float32)
    previous_usage = np.asarray(previous_usage, np.float32)
    write_weighting = np.asarray(write_weighting, np.float32)

    # ---- host prep: shared small tensors ---------------------------------
    kn = max(float(np.linalg.norm(desired_content)), EPS)
    scale = np.float32(float(key_strength[0]) / kn)
    kh = (desired_content * scale).astype(np.float16)
    skd = np.zeros((128, NT, 32), np.float16)
    for t in range(NT):
        skd[0:64, t, 2 * t] = kh
        skd[64:128, t, 2 * t + 1] = kh
    skd = np.ascontiguousarray(skd.reshape(128, NT * 32))
    # indicator: variant v, partition half hblk, k-tile j -> col 4v + 2*hblk + j
    sk8 = np.zeros((128, 2, 8 * 32), np.float32)
    for v in range(8):
        for hblk in range(2):
            for j in range(2):
                sk8[64 * hblk : 64 * hblk + 64, j, 32 * v + 4 * v + 2 * hblk + j] = 1.0
    sk8 = sk8.astype(ml_dtypes.float8_e4m3)
    negf = np.tile(-free_gate.astype(np.float32), (128, 1))

    # ---- host prep: per-core shards --------------------------------------
    in_maps = []
    for c in range(NCORES):
        sl = slice(c * RPC, (c + 1) * RPC)
        shard = memory[sl]
        mt = np.empty((128, HALF), np.float16)
        mt[:64] = shard[:HALF].T
        mt[64:] = shard[HALF:].T
        rw = read_weighting[sl]
        rwt = np.empty((128, R * 1024), np.float16)
        for h in range(R):
            rwt[:, h * 1024 : (h + 1) * 1024] = rw[:, h].reshape(128, 1024)
        in_maps.append(
            {
                "mt": mt,
                "skd": skd,
                "sk8": sk8,
                "negf": negf,
                "rwt": rwt,
                "prev": previous_usage[sl].reshape(128, 1024).astype(np.float16),
                "wr": write_weighting[sl].reshape(128, 1024).astype(np.float16),
            }
        )

    # ---- run on the 8 NeuronCores ----------------------------------------
    trace = os.environ.get("BASS_TRACE", "") not in ("", "0")
    if trace:
        _install_ntff_hook()
    nc = _get_nc()
    reps = int(os.environ.get("BASS_REPEAT", "1"))
    times = []
    for rep in range(reps):
        res = run_bass_kernel_spmd(
            nc,
            in_maps,
            core_ids=list(range(NCORES)),
            trace=trace,
            tmpdir=(os.environ.get("BASS_TRACE_DIR") or None) if reps == 1 else None,
        )
        if res.exec_time_ns is not None:
            times.append(res.exec_time_ns)
    LAST["exec_time_ns"] = min(times) if times else None
    LAST["exec_times"] = times
    LAST["results"] = res

    # ---- gather / unshard -------------------------------------------------
    pnum = np.concatenate(
        [np.asarray(r["p_out"], np.float32).reshape(-1) for r in res.results]
    )
    retention = np.concatenate(
        [np.asarray(r["ret_out"], np.float32).reshape(-1) for r in res.results]
    )
    usage = np.concatenate(
        [np.asarray(r["use_out"], np.float32).reshape(-1) for r in res.results]
    )
    esum = np.concatenate(
        [np.asarray(r["esum_out"], np.float32).reshape(-1) for r in res.results]
    )
    S = np.sum(esum, dtype=np.float32)
    content = (pnum / S).astype(np.float32)

    allocation = _allocation_weighting(
        usage, previous_usage, write_weighting, read_weighting, free_gate
    )

    return np.stack([content, retention, usage, allocation]).astype(np.float32)


def _allocation_weighting(usage_approx, prev, wr, rw, fg) -> np.ndarray:
    """Faithful f32 replica of the reference allocation computation.

    The device usage is fp16-approximate; the ordering of the smallest
    usages decides allocation, so the K candidate slots (found from the
    approximate values) are recomputed exactly in f32 from the original
    inputs, matching the reference op-for-op:
      ret = prod_r (1 - rw*fg);  u = (p + w - p*w) * ret
      idx = stable argsort(u); s = u[idx]
      alloc_sorted[j] = (1 - s[max(j-1,0)]) * prod_{i<j} s[i]
    The exclusive cumprod of ascending f32 values in [0,1) underflows to
    exact 0 within a few dozen terms, so only the K smallest slots matter.
    """
    n = usage_approx.shape[0]
    K = min(4096, n)
    cand = np.argpartition(usage_approx, K - 1)[:K]
    retc = np.prod(
        (np.float32(1.0) - rw[cand] * fg[None, :]).astype(np.float32),
        axis=1, dtype=np.float32,
    )
    p, w = prev[cand], wr[cand]
    uc = ((p + w) - p * w).astype(np.float32) * retc
    order = np.lexsort((cand, uc))  # by exact value, ties by index (stable)
    sidx = cand[order]
    s = uc[order]
    excl = np.empty(K, np.float32)
    excl[0] = np.float32(1.0)
    np.cumprod(s[:-1], dtype=np.float32, out=excl[1:])
    if K < n and excl[-1] != 0.0:
        # cumprod did not underflow within K terms: fall back to full exact
        # recompute + sort over all N
        retf = np.prod(
            (np.float32(1.0) - rw * fg[None, :]).astype(np.float32),
            axis=1, dtype=np.float32,
        )
        uf = ((prev + wr) - prev * wr).astype(np.float32) * retf
        sidx = np.argsort(uf, kind="stable")
        s = uf[sidx]
        excl = np.concatenate(
            [[np.float32(1.0)], np.cumprod(s[:-1], dtype=np.float32)]
        ).astype(np.float32)
    shifted = np.concatenate([s[:1], s[:-1]])
    alloc_sorted = ((np.float32(1.0) - shifted) * excl).astype(np.float32)
    allocation = np.zeros(n, np.float32)
    allocation[sidx] = alloc_sorted
    return allocation
